# revision 57
# baseline (speedup 1.0000x reference)
"""GNN message-passing layer on 8 TRN2 NeuronCores.

Math: y[e] = relu(concat(x[i[e]], x[i[e]]) @ W1 + b1) @ W2 + b2
         = relu(x[i[e]] @ (W1[:C]+W1[C:]) + b1) @ W2 + b2.
The MLP depends only on the source node, so compute z = MLP(x) once per
node (50k rows), then y = z[nbr_idx] is a pure gather (800k rows).

Sharding: edges are split evenly across the 8 cores; each core computes
the full z table locally (x + weights replicated; phase A is tiny) and
then gathers + writes its own edge shard. No collectives.

Phase A: h^T = relu(W1eff^T x^T + b1) in column form (512-node moving
chunks, W1 stationary), then z in ROW form directly: one matmul per
128-node group with a stride-GRP stationary slice of h^T, so group r
of a super-chunk holds nodes {GRP*p + r} and partition p's zbuf row is
GRP consecutive z rows -> multi-KB contiguous DRAM writes. No PE
transposes; b2 is added on the host (z gathers commute with +b2).
Super-chunks are 4096 nodes (PE bursts long enough to ramp the tensor
engine to full clock) with smaller first/last chunks to shorten the
pipeline fill and drain; the row-form matmuls of each chunk are
interleaved between the column-form matmuls of the next.

Phase B: GPSIMD dma_gather at pair-row granularity (signed int16
indices only reach 32767, so the bf16 z table is gathered as 512B rows
of 2 nodes; pair id < 25088 fits int16). A DVE predicated copy selects
the right half per edge (mask = idx & 1), a second DVE copy compacts to
a dense tile, and y is written bf16 with 4KB runs per partition. The
host upcasts to f32 (identical values to an on-chip upcast) and adds
b2.
"""

from contextlib import ExitStack

import ml_dtypes
import numpy as np

import concourse.bacc as bacc
import concourse.mybir as mybir
import concourse.tile as tile
from concourse import library_config
from concourse.bass_utils import run_bass_kernel_spmd

N_CORES = 8
C = 128  # channels (C_IN == C_OUT)
N_NODES = 50000
E_TOTAL = 800000

ACH = 512  # phase-A compute chunk (max moving dim per matmul)
SCH = 2048  # phase-A DMA super-chunk (one x load + one z write)
NPAD = ((N_NODES + 511) // 512) * 512  # 50176
GRP = SCH // 128  # 16 row-form matmul groups per super-chunk

import os
EPC = E_TOTAL // N_CORES  # 100000 edges per core
NI = 2048  # edges per gather tile
TBB = (EPC + NI - 1) // NI  # 49 gather tiles
EPC_PAD = TBB * NI  # 100352
KCH = NI // 128  # 16 gathered rows per partition per tile
# phase-B tiles: uniform 2048-edge tiles with two 1024-edge tail tiles to
# shorten the end-of-kernel drain (gather -> select -> write chain)
def _btiles():
    # graded: small head tiles so the first gather's descriptor generation
    # (and thus the first post-barrier DMA) starts sooner; small tail tiles
    # to shorten the end-of-kernel gather->select->write drain
    sizes = [NI] * 48 + [1024, 1024]
    assert sum(sizes) == EPC_PAD
    out, off = [], 0
    for ni in sizes:
        out.append((off, ni))
        off += ni
    return out


BTILES = _btiles()

F32 = mybir.dt.float32
BF16 = mybir.dt.bfloat16

# matmul input dtype for phase A
MM_DT = mybir.dt.bfloat16


PHASES = os.environ.get("KPHASES", "AB")


def _build_nc():
    nc = bacc.Bacc("TRN2", target_bir_lowering=False, debug=False,
                   num_devices=N_CORES, dynamic_dma_scratch_size=65536)

    xT = nc.dram_tensor("xT", [C, NPAD], BF16, kind="ExternalInput")
    idx16 = nc.dram_tensor("idx16", [128, EPC_PAD // 16], mybir.dt.int16,
                           kind="ExternalInput")
    parity = nc.dram_tensor("parity", [128, EPC_PAD // 128], mybir.dt.uint8,
                            kind="ExternalInput")
    w1 = nc.dram_tensor("w1", [C, C], BF16, kind="ExternalInput")
    w2 = nc.dram_tensor("w2", [C, C], BF16, kind="ExternalInput")
    b1 = nc.dram_tensor("b1", [C, 1], F32, kind="ExternalInput")
    y = nc.dram_tensor("y", [EPC_PAD, C], BF16, kind="ExternalOutput")
    zkind = "ExternalOutput" if PHASES == "A" else \
        ("ExternalInput" if PHASES == "B" else "Internal")
    z = nc.dram_tensor("z_table", [NPAD, C], BF16, kind=zkind)

    with tile.TileContext(nc) as tc, ExitStack() as ctx:
        const = ctx.enter_context(tc.tile_pool(name="const", bufs=1))
        xpool = ctx.enter_context(tc.tile_pool(name="xin", bufs=4))
        hpool = ctx.enter_context(tc.tile_pool(name="hbuf", bufs=3))
        zb_pool = ctx.enter_context(tc.tile_pool(name="zb", bufs=4))
        gpool = ctx.enter_context(tc.tile_pool(name="gbuf", bufs=5))
        spool = ctx.enter_context(tc.tile_pool(name="sel", bufs=4))
        psA = ctx.enter_context(tc.tile_pool(name="psA", bufs=4, space="PSUM"))
        psB = ctx.enter_context(tc.tile_pool(name="psB", bufs=4, space="PSUM"))

        w1t = const.tile([C, C], MM_DT)
        w2t = const.tile([C, C], MM_DT)
        b1t = const.tile([C, 1], F32)
        idxt = const.tile([128, EPC_PAD // 16], mybir.dt.int16)
        maskt = const.tile([128, EPC_PAD // 128], mybir.dt.uint8)
        if "B" in PHASES:
            nc.gpsimd.load_library(library_config.mlp)
        nc.scalar.dma_start(out=w1t[:], in_=w1[:])
        nc.scalar.dma_start(out=b1t[:], in_=b1[:])
        nc.scalar.dma_start(out=w2t[:], in_=w2[:])
        nc.scalar.dma_start(out=idxt[:], in_=idx16[:])
        nc.scalar.dma_start(out=maskt[:], in_=parity[:])

        # ---- Phase A (skipped when PHASES=="B"). Emission is software-
        # pipelined one super-chunk deep, and the row-form matmuls of the
        # previous chunk are interleaved between the column-form matmuls
        # of the current chunk so the PE streams without engine gaps
        # (mm1 PSUM banks drain on ACT while the PE runs mm2s). 4096-node
        # super-chunks keep each PE burst long enough to ramp the tensor
        # engine to its full p-state clock.
        SC1 = 4096  # big chunks keep PE bursts long enough to reach full clock
        chunks = []
        if "A" in PHASES:
            # graded sizes: small first chunk to hide the initial x DMA
            # latency, small final chunks to shorten the pipeline drain
            sizes = [512, 1536] + [SC1] * 11 + [2048, 512, 512]
            assert sum(sizes) == NPAD
            n0 = 0
            for sch in sizes:
                chunks.append((n0, sch))
                n0 += sch

        def zout(n0, grp, zbuf, qlo, qhi):
            # rows {grp*p + r}: contiguous per-partition runs
            nc.sync.dma_start(
                out=z[n0:n0 + grp * 128, :].rearrange(
                    "(p r) c -> p r c", r=grp)[:, qlo * 4:qhi * 4, :],
                in_=zbuf[:, qlo * 4:qhi * 4, :])

        def chunk(n0, sch, prev, late=False):
            xt = xpool.tile([C, SC1], MM_DT, tag="xt")
            nc.sync.dma_start(out=xt[:, 0:sch], in_=xT[:, n0:n0 + sch])
            h_all = hpool.tile([C, SC1], MM_DT, tag="h")
            zbuf = None
            if prev is not None:
                h_prev, n0p, schp = prev
                grpp = schp // 128
                zbuf = zb_pool.tile([128, SC1 // 128, C], BF16, tag="zbuf")
            nb = sch // ACH
            for b in range(nb):
                h_ps = psA.tile([C, ACH], F32, tag="h_ps")
                nc.tensor.matmul(h_ps[:], w1t[:],
                                 xt[:, b * ACH:(b + 1) * ACH],
                                 start=True, stop=True)
                if prev is not None:
                    qlo = (grpp * b) // (4 * nb)
                    qhi = (grpp * (b + 1)) // (4 * nb)
                    for q in range(qlo, qhi):
                        z_ps = psB.tile([128, 4, C], F32, tag="z_ps")
                        for j in range(4):
                            r = q * 4 + j
                            nc.tensor.matmul(z_ps[:, j, :],
                                             h_prev[:, r:schp:grpp], w2t[:],
                                             start=True, stop=True)
                        if late and q % 2 == 1:
                            nc.scalar.copy(zbuf[:, q * 4:(q + 1) * 4, :],
                                           z_ps[:])
                        else:
                            nc.vector.tensor_copy(
                                zbuf[:, q * 4:(q + 1) * 4, :], z_ps[:])
                nc.scalar.activation(h_all[:, b * ACH:(b + 1) * ACH], h_ps[:],
                                     mybir.ActivationFunctionType.Relu,
                                     bias=b1t[:, 0:1])
            if prev is not None:
                zout(n0p, grpp, zbuf, 0, grpp // 4)
            return (h_all, n0, sch)

        def tailchunk(prev):
            h_prev, n0, sch = prev
            grp = sch // 128
            zbuf = zb_pool.tile([128, SC1 // 128, C], BF16, tag="zbuf")
            for q in range(grp // 4):
                z_ps = psB.tile([128, 4, C], F32, tag="z_ps")
                for j in range(4):
                    r = q * 4 + j
                    nc.tensor.matmul(z_ps[:, j, :], h_prev[:, r:sch:grp],
                                     w2t[:], start=True, stop=True)
                nc.scalar.copy(zbuf[:, q * 4:(q + 1) * 4, :], z_ps[:])
                # write as soon as computed to overlap the pipeline drain
                zout(n0, grp, zbuf, q, q + 1)

        prev = None
        for ci, (n0, sch) in enumerate(chunks):
            prev = chunk(n0, sch, prev, late=(ci >= len(chunks) - 3))
        if prev is not None:
            tailchunk(prev)

        zview = z[:].rearrange("(a two) c -> a (two c)", two=2)  # [NPAD/2,2C]

        # No explicit barrier between the phases: the gathers' reads of the
        # z table carry tracked RAW sync deps on all of phase A's z writes,
        # which is the exact ordering required (verified: without it phase B
        # still starts only after the final z write lands).

        # ---- Phase B: dma_gather pair rows, DVE half-select + compaction,
        # coalesced bf16 write. Edge e = t*NI + p*KCH + k sits at SBUF
        # [p, k, :] so each partition writes one contiguous 4KB run of y
        # rows per tile.
        for t, (off, ni) in enumerate(BTILES if "B" in PHASES else []):
            ki = ni // 128
            g = gpool.tile([128, KCH, 2 * C], BF16, tag="g")
            nc.gpsimd.dma_gather(
                out_ap=g[:, 0:ki, :], in_ap=zview,
                idxs_ap=idxt[:, off // 16:(off + ni) // 16],
                num_idxs=ni, num_idxs_reg=ni, elem_size=2 * C,
                single_packet=False)
            even = g[:, 0:ki, 0:C]
            odd = g[:, 0:ki, C:2 * C]
            mo = off // 128
            m = maskt[:, mo:mo + ki].to_broadcast([128, ki, C])
            nc.vector.copy_predicated(out=even, mask=m, data=odd)
            sel = spool.tile([128, KCH, C], BF16, tag="sel")
            nc.vector.tensor_copy(sel[:, 0:ki, :], even)
            # alternate the two HWDGE rings for the big y writes
            weng = nc.sync if t % 2 == 0 else nc.scalar
            weng.dma_start(
                out=y[off:off + ni, :].rearrange("(p k) c -> p k c", k=ki),
                in_=sel[:, 0:ki, :])

    nc.compile()
    return nc


def _pack_indices(idx_pad):
    """idx_pad: int32 [EPC_PAD] -> (idx16 [128, EPC_PAD//16] int16,
    parity [128, EPC_PAD//128] uint8). Within a tile of ni edges, edge row
    r sits at gather position i = (r%ki)*128 + r//ki (ki = ni//128), so it
    lands at out [r//ki, r%ki, :]; gather positions wrap into 16 partitions
    ([i%16, i//16]), replicated across the 8 GPSIMD cores."""
    pair = (idx_pad >> 1).astype(np.int16)
    par = (idx_pad & 1).astype(np.uint8)

    idx_cols, mask_cols = [], []
    for off, ni in BTILES:
        ki = ni // 128
        r = np.arange(ni)
        pos = (r % ki) * 128 + r // ki
        pb = np.empty(ni, dtype=np.int16)
        pb[pos] = pair[off:off + ni]
        idx_cols.append(pb.reshape(ni // 16, 16).T)
        mask_cols.append(par[off:off + ni].reshape(128, ki))
    idx16 = np.tile(np.ascontiguousarray(np.concatenate(idx_cols, axis=1)),
                    (8, 1))
    mask = np.ascontiguousarray(np.concatenate(mask_cols, axis=1))
    return idx16, mask


_NC_CACHE = None


def _get_nc():
    global _NC_CACHE
    if _NC_CACHE is None:
        _NC_CACHE = _build_nc()
    return _NC_CACHE


def kernel(x, nbr_idx, W1, b1, W2, b2, _trace=False, _trace_kwargs=None):
    x = np.asarray(x, dtype=np.float32)
    nbr_idx_np = np.asarray(nbr_idx)
    W1 = np.asarray(W1, dtype=np.float32)
    W2 = np.asarray(W2, dtype=np.float32)
    b1 = np.asarray(b1, dtype=np.float32)
    b2 = np.asarray(b2, dtype=np.float32)

    w1eff = np.ascontiguousarray(W1[:C] + W1[C:]).astype(ml_dtypes.bfloat16)
    w2_bf = W2.astype(ml_dtypes.bfloat16)
    xT = np.zeros((C, NPAD), dtype=ml_dtypes.bfloat16)
    xT[:, :N_NODES] = x.T.astype(ml_dtypes.bfloat16)

    in_maps = []
    for i in range(N_CORES):
        idx_pad = np.zeros(EPC_PAD, dtype=np.int32)
        idx_pad[:EPC] = nbr_idx_np[i * EPC:(i + 1) * EPC].astype(np.int32)
        idx16, mask = _pack_indices(idx_pad)
        in_maps.append({
            "xT": xT,
            "idx16": idx16,
            "parity": mask,
            "w1": w1eff,
            "w2": w2_bf,
            "b1": b1.reshape(C, 1),
        })

    nc = _get_nc()
    res = run_bass_kernel_spmd(nc, in_maps, list(range(N_CORES)),
                               trace=_trace, **(_trace_kwargs or {}))

    b2f = b2.astype(np.float32)
    out = np.empty((E_TOTAL, C), dtype=np.float32)
    for i in range(N_CORES):
        out[i * EPC:(i + 1) * EPC] = (
            res.results[i]["y"][:EPC].astype(np.float32) + b2f)
    if _trace:
        return out, res
    return out


# revision 58
# speedup vs baseline: 1.0019x; 1.0019x over previous
"""GNN message-passing layer on 8 TRN2 NeuronCores.

Math: y[e] = relu(concat(x[i[e]], x[i[e]]) @ W1 + b1) @ W2 + b2
         = relu(x[i[e]] @ (W1[:C]+W1[C:]) + b1) @ W2 + b2.
The MLP depends only on the source node, so compute z = MLP(x) once per
node (50k rows), then y = z[nbr_idx] is a pure gather (800k rows).

Sharding: edges are split evenly across the 8 cores; each core computes
the full z table locally (x + weights replicated; phase A is tiny) and
then gathers + writes its own edge shard. No collectives.

Phase A: h^T = relu(W1eff^T x^T + b1) in column form (512-node moving
chunks, W1 stationary), then z in ROW form directly: one matmul per
128-node group with a stride-GRP stationary slice of h^T, so group r
of a super-chunk holds nodes {GRP*p + r} and partition p's zbuf row is
GRP consecutive z rows -> multi-KB contiguous DRAM writes. No PE
transposes; b2 is added on the host (z gathers commute with +b2).
Super-chunks are 4096 nodes (PE bursts long enough to ramp the tensor
engine to full clock) with smaller first/last chunks to shorten the
pipeline fill and drain; the row-form matmuls of each chunk are
interleaved between the column-form matmuls of the next.

Phase B: GPSIMD dma_gather at pair-row granularity (signed int16
indices only reach 32767, so the bf16 z table is gathered as 512B rows
of 2 nodes; pair id < 25088 fits int16). A DVE predicated copy selects
the right half per edge (mask = idx & 1), a second DVE copy compacts to
a dense tile, and y is written bf16 with 4KB runs per partition. The
host upcasts to f32 (identical values to an on-chip upcast) and adds
b2.
"""

from contextlib import ExitStack

import ml_dtypes
import numpy as np

import concourse.bacc as bacc
import concourse.mybir as mybir
import concourse.tile as tile
from concourse import library_config
from concourse.bass_utils import run_bass_kernel_spmd

N_CORES = 8
C = 128  # channels (C_IN == C_OUT)
N_NODES = 50000
E_TOTAL = 800000

ACH = 512  # phase-A compute chunk (max moving dim per matmul)
SCH = 2048  # phase-A DMA super-chunk (one x load + one z write)
NPAD = ((N_NODES + 511) // 512) * 512  # 50176
GRP = SCH // 128  # 16 row-form matmul groups per super-chunk

import os
EPC = E_TOTAL // N_CORES  # 100000 edges per core
NI = 2048  # edges per gather tile
TBB = (EPC + NI - 1) // NI  # 49 gather tiles
EPC_PAD = ((EPC + 127) // 128) * 128  # 100096: minimal 128-slot coverage
KCH = NI // 128  # 16 gathered rows per partition per tile
# phase-B tiles: uniform 2048-edge tiles with two 1024-edge tail tiles to
# shorten the end-of-kernel drain (gather -> select -> write chain)
def _btiles():
    # graded: small head tiles so the first gather's descriptor generation
    # (and thus the first post-barrier DMA) starts sooner; small tail tiles
    # to shorten the end-of-kernel gather->select->write drain
    sizes = [NI] * 48 + [1024, 768]
    assert sum(sizes) == EPC_PAD
    out, off = [], 0
    for ni in sizes:
        out.append((off, ni))
        off += ni
    return out


BTILES = _btiles()

F32 = mybir.dt.float32
BF16 = mybir.dt.bfloat16

# matmul input dtype for phase A
MM_DT = mybir.dt.bfloat16


PHASES = os.environ.get("KPHASES", "AB")


def _build_nc():
    nc = bacc.Bacc("TRN2", target_bir_lowering=False, debug=False,
                   num_devices=N_CORES, dynamic_dma_scratch_size=65536)

    xT = nc.dram_tensor("xT", [C, NPAD], BF16, kind="ExternalInput")
    idx16 = nc.dram_tensor("idx16", [128, EPC_PAD // 16], mybir.dt.int16,
                           kind="ExternalInput")
    parity = nc.dram_tensor("parity", [128, EPC_PAD // 128], mybir.dt.uint8,
                            kind="ExternalInput")
    w1 = nc.dram_tensor("w1", [C, C], BF16, kind="ExternalInput")
    w2 = nc.dram_tensor("w2", [C, C], BF16, kind="ExternalInput")
    b1 = nc.dram_tensor("b1", [C, 1], F32, kind="ExternalInput")
    y = nc.dram_tensor("y", [EPC_PAD, C], BF16, kind="ExternalOutput")
    zkind = "ExternalOutput" if PHASES == "A" else \
        ("ExternalInput" if PHASES == "B" else "Internal")
    z = nc.dram_tensor("z_table", [NPAD, C], BF16, kind=zkind)

    with tile.TileContext(nc) as tc, ExitStack() as ctx:
        const = ctx.enter_context(tc.tile_pool(name="const", bufs=1))
        xpool = ctx.enter_context(tc.tile_pool(name="xin", bufs=4))
        hpool = ctx.enter_context(tc.tile_pool(name="hbuf", bufs=3))
        zb_pool = ctx.enter_context(tc.tile_pool(name="zb", bufs=4))
        gpool = ctx.enter_context(tc.tile_pool(name="gbuf", bufs=5))
        spool = ctx.enter_context(tc.tile_pool(name="sel", bufs=4))
        psA = ctx.enter_context(tc.tile_pool(name="psA", bufs=4, space="PSUM"))
        psB = ctx.enter_context(tc.tile_pool(name="psB", bufs=4, space="PSUM"))

        w1t = const.tile([C, C], MM_DT)
        w2t = const.tile([C, C], MM_DT)
        b1t = const.tile([C, 1], F32)
        idxt = const.tile([128, EPC_PAD // 16], mybir.dt.int16)
        maskt = const.tile([128, EPC_PAD // 128], mybir.dt.uint8)
        if "B" in PHASES:
            nc.gpsimd.load_library(library_config.mlp)
        nc.scalar.dma_start(out=w1t[:], in_=w1[:])
        nc.scalar.dma_start(out=b1t[:], in_=b1[:])
        nc.scalar.dma_start(out=w2t[:], in_=w2[:])
        nc.scalar.dma_start(out=idxt[:], in_=idx16[:])
        nc.scalar.dma_start(out=maskt[:], in_=parity[:])

        # ---- Phase A (skipped when PHASES=="B"). Emission is software-
        # pipelined one super-chunk deep, and the row-form matmuls of the
        # previous chunk are interleaved between the column-form matmuls
        # of the current chunk so the PE streams without engine gaps
        # (mm1 PSUM banks drain on ACT while the PE runs mm2s). 4096-node
        # super-chunks keep each PE burst long enough to ramp the tensor
        # engine to its full p-state clock.
        SC1 = 4096  # big chunks keep PE bursts long enough to reach full clock
        chunks = []
        if "A" in PHASES:
            # graded sizes: small first chunk to hide the initial x DMA
            # latency, small final chunks to shorten the pipeline drain
            sizes = [512, 1536] + [SC1] * 11 + [2048, 512, 512]
            assert sum(sizes) == NPAD
            n0 = 0
            for sch in sizes:
                chunks.append((n0, sch))
                n0 += sch

        def zout(n0, grp, zbuf, qlo, qhi):
            # rows {grp*p + r}: contiguous per-partition runs
            nc.sync.dma_start(
                out=z[n0:n0 + grp * 128, :].rearrange(
                    "(p r) c -> p r c", r=grp)[:, qlo * 4:qhi * 4, :],
                in_=zbuf[:, qlo * 4:qhi * 4, :])

        def chunk(n0, sch, prev, late=False):
            xt = xpool.tile([C, SC1], MM_DT, tag="xt")
            nc.sync.dma_start(out=xt[:, 0:sch], in_=xT[:, n0:n0 + sch])
            h_all = hpool.tile([C, SC1], MM_DT, tag="h")
            zbuf = None
            if prev is not None:
                h_prev, n0p, schp = prev
                grpp = schp // 128
                zbuf = zb_pool.tile([128, SC1 // 128, C], BF16, tag="zbuf")
            nb = sch // ACH
            for b in range(nb):
                h_ps = psA.tile([C, ACH], F32, tag="h_ps")
                nc.tensor.matmul(h_ps[:], w1t[:],
                                 xt[:, b * ACH:(b + 1) * ACH],
                                 start=True, stop=True)
                if prev is not None:
                    qlo = (grpp * b) // (4 * nb)
                    qhi = (grpp * (b + 1)) // (4 * nb)
                    for q in range(qlo, qhi):
                        z_ps = psB.tile([128, 4, C], F32, tag="z_ps")
                        for j in range(4):
                            r = q * 4 + j
                            nc.tensor.matmul(z_ps[:, j, :],
                                             h_prev[:, r:schp:grpp], w2t[:],
                                             start=True, stop=True)
                        if late and q % 2 == 1:
                            nc.scalar.copy(zbuf[:, q * 4:(q + 1) * 4, :],
                                           z_ps[:])
                        else:
                            nc.vector.tensor_copy(
                                zbuf[:, q * 4:(q + 1) * 4, :], z_ps[:])
                nc.scalar.activation(h_all[:, b * ACH:(b + 1) * ACH], h_ps[:],
                                     mybir.ActivationFunctionType.Relu,
                                     bias=b1t[:, 0:1])
            if prev is not None:
                zout(n0p, grpp, zbuf, 0, grpp // 4)
            return (h_all, n0, sch)

        def tailchunk(prev):
            h_prev, n0, sch = prev
            grp = sch // 128
            zbuf = zb_pool.tile([128, SC1 // 128, C], BF16, tag="zbuf")
            for q in range(grp // 4):
                z_ps = psB.tile([128, 4, C], F32, tag="z_ps")
                for j in range(4):
                    r = q * 4 + j
                    nc.tensor.matmul(z_ps[:, j, :], h_prev[:, r:sch:grp],
                                     w2t[:], start=True, stop=True)
                nc.scalar.copy(zbuf[:, q * 4:(q + 1) * 4, :], z_ps[:])
                # write as soon as computed to overlap the pipeline drain
                zout(n0, grp, zbuf, q, q + 1)

        prev = None
        for ci, (n0, sch) in enumerate(chunks):
            prev = chunk(n0, sch, prev, late=(ci >= len(chunks) - 3))
        if prev is not None:
            tailchunk(prev)

        zview = z[:].rearrange("(a two) c -> a (two c)", two=2)  # [NPAD/2,2C]

        # No explicit barrier between the phases: the gathers' reads of the
        # z table carry tracked RAW sync deps on all of phase A's z writes,
        # which is the exact ordering required (verified: without it phase B
        # still starts only after the final z write lands).

        # ---- Phase B: dma_gather pair rows, DVE half-select + compaction,
        # coalesced bf16 write. Edge e = t*NI + p*KCH + k sits at SBUF
        # [p, k, :] so each partition writes one contiguous 4KB run of y
        # rows per tile.
        for t, (off, ni) in enumerate(BTILES if "B" in PHASES else []):
            ki = ni // 128
            g = gpool.tile([128, KCH, 2 * C], BF16, tag="g")
            nc.gpsimd.dma_gather(
                out_ap=g[:, 0:ki, :], in_ap=zview,
                idxs_ap=idxt[:, off // 16:(off + ni) // 16],
                num_idxs=ni, num_idxs_reg=ni, elem_size=2 * C,
                single_packet=False)
            even = g[:, 0:ki, 0:C]
            odd = g[:, 0:ki, C:2 * C]
            mo = off // 128
            m = maskt[:, mo:mo + ki].to_broadcast([128, ki, C])
            nc.vector.copy_predicated(out=even, mask=m, data=odd)
            sel = spool.tile([128, KCH, C], BF16, tag="sel")
            nc.vector.tensor_copy(sel[:, 0:ki, :], even)
            # alternate the two HWDGE rings for the big y writes
            weng = nc.sync if t % 2 == 0 else nc.scalar
            weng.dma_start(
                out=y[off:off + ni, :].rearrange("(p k) c -> p k c", k=ki),
                in_=sel[:, 0:ki, :])

    nc.compile()
    return nc


def _pack_indices(idx_pad):
    """idx_pad: int32 [EPC_PAD] -> (idx16 [128, EPC_PAD//16] int16,
    parity [128, EPC_PAD//128] uint8). Within a tile of ni edges, edge row
    r sits at gather position i = (r%ki)*128 + r//ki (ki = ni//128), so it
    lands at out [r//ki, r%ki, :]; gather positions wrap into 16 partitions
    ([i%16, i//16]), replicated across the 8 GPSIMD cores."""
    pair = (idx_pad >> 1).astype(np.int16)
    par = (idx_pad & 1).astype(np.uint8)

    idx_cols, mask_cols = [], []
    for off, ni in BTILES:
        ki = ni // 128
        r = np.arange(ni)
        pos = (r % ki) * 128 + r // ki
        pb = np.empty(ni, dtype=np.int16)
        pb[pos] = pair[off:off + ni]
        idx_cols.append(pb.reshape(ni // 16, 16).T)
        mask_cols.append(par[off:off + ni].reshape(128, ki))
    idx16 = np.tile(np.ascontiguousarray(np.concatenate(idx_cols, axis=1)),
                    (8, 1))
    mask = np.ascontiguousarray(np.concatenate(mask_cols, axis=1))
    return idx16, mask


_NC_CACHE = None


def _get_nc():
    global _NC_CACHE
    if _NC_CACHE is None:
        _NC_CACHE = _build_nc()
    return _NC_CACHE


def kernel(x, nbr_idx, W1, b1, W2, b2, _trace=False, _trace_kwargs=None):
    x = np.asarray(x, dtype=np.float32)
    nbr_idx_np = np.asarray(nbr_idx)
    W1 = np.asarray(W1, dtype=np.float32)
    W2 = np.asarray(W2, dtype=np.float32)
    b1 = np.asarray(b1, dtype=np.float32)
    b2 = np.asarray(b2, dtype=np.float32)

    w1eff = np.ascontiguousarray(W1[:C] + W1[C:]).astype(ml_dtypes.bfloat16)
    w2_bf = W2.astype(ml_dtypes.bfloat16)
    xT = np.zeros((C, NPAD), dtype=ml_dtypes.bfloat16)
    xT[:, :N_NODES] = x.T.astype(ml_dtypes.bfloat16)

    in_maps = []
    for i in range(N_CORES):
        idx_pad = np.zeros(EPC_PAD, dtype=np.int32)
        idx_pad[:EPC] = nbr_idx_np[i * EPC:(i + 1) * EPC].astype(np.int32)
        idx16, mask = _pack_indices(idx_pad)
        in_maps.append({
            "xT": xT,
            "idx16": idx16,
            "parity": mask,
            "w1": w1eff,
            "w2": w2_bf,
            "b1": b1.reshape(C, 1),
        })

    nc = _get_nc()
    res = run_bass_kernel_spmd(nc, in_maps, list(range(N_CORES)),
                               trace=_trace, **(_trace_kwargs or {}))

    b2f = b2.astype(np.float32)
    out = np.empty((E_TOTAL, C), dtype=np.float32)
    for i in range(N_CORES):
        out[i * EPC:(i + 1) * EPC] = (
            res.results[i]["y"][:EPC].astype(np.float32) + b2f)
    if _trace:
        return out, res
    return out
